# revision 13
# baseline (speedup 1.0000x reference)
"""Trainium2 Bass kernel: cosine-similarity message passing (GNN aggregate).

Math (collapsed — the [N,N] similarity matrix is never materialized):
    x_hat = x / max(||x||, eps)                      row-normalized features
    G'    = x_hat.T @ [x | 1]        [D, D+1]        Gram + column-sum s
    oa    = x @ G'                   [N, D+1]        (query-side normalization
                                                      cancels in the ratio)
    out   = oa[:, :D] / oa[:, D:D+1]

Sharding (v2): G' is a sum over ALL rows, so each core computes a partial
G' from only its OWN 1/8 row block (1 MB read instead of the baseline's
redundant 8 MB full-x stream), and an 8-core AllReduce of the [D, D+1]
partials (263 KB fp32) combines them. Phase 2 (own rows x G', divide by
row sum) is unchanged. Per-core HBM traffic drops 10 MB -> ~2.5 MB; the
collective (~13 us) dominates the steady-state per-iteration time.

Environment quirks encoded here:
 - this walrus build accepts at most ONE sync wait per instruction:
   _legalize_sync_waits hoists extras onto same-engine Drain carriers
   (wired via nc.to_json_bytes); tensor_tensor_reduce doesn't compile.
 - the ACT sqrt spline is near-exact on this HW (a Newton refinement
   measurably HURT accuracy in the baseline session).
 - eps in max(||x||, eps) never binds for gaussian rows (min norm ~14).
"""

import numpy as np
from contextlib import ExitStack

import concourse.bass as bass
import concourse.tile as tile
from concourse import mybir
from concourse.masks import make_identity
from concourse.bass_utils import run_bass_kernel_spmd

N, D = 8192, 256
NCORES = 8
P = 128
OWN = N // NCORES            # 1024 rows per core
OWN_T = OWN // P             # 8 own tiles
DA = D + 1                   # 257: x columns + ones column
F32 = mybir.dt.float32
BF16 = mybir.dt.bfloat16
AF = mybir.ActivationFunctionType

# engine split knobs: tiles j < ACT_P1_J do square+reduce on ScalarE, the
# rest on VectorE; tiles j < GPS_P2_J scale on GpSimd, the rest on VectorE.
ACT_P1_J = 4
GPS_P2_J = 6

_nc_cache = {}


def _legalize_sync_waits(bir_bytes: bytes) -> bytes:
    """This walrus build accepts at most ONE sync wait per instruction.
    Tile emits several; hoist the extras onto same-engine Drain
    instructions placed immediately before (queue order preserves the
    semantics of inline waits)."""
    import orjson
    bir = orjson.loads(bir_bytes)
    ctr = [0]

    def fix_block(blk):
        new_list = []
        for ins in blk.get("instructions", []):
            si = ins.get("sync_info")
            if si:
                waits = si.get("on_wait") or []
                if len(waits) > 1:
                    for w in waits[:-1]:
                        ctr[0] += 1
                        new_list.append({
                            "debug": ins.get("debug", 0),
                            "engine": ins["engine"],
                            "ins": [], "outs": [],
                            "name": f"I-lw{ctr[0]}",
                            "opcode": "Drain",
                            "sync_info": {"on_update": [], "on_wait": [w]},
                        })
                    si["on_wait"] = waits[-1:]
            new_list.append(ins)
        blk["instructions"] = new_list
        for sb in blk.get("blocks", []):
            fix_block(sb)

    for f in bir["functions"]:
        for blk in f["blocks"]:
            fix_block(blk)
    return orjson.dumps(bir)


def _build_nc(iters: int = 1):
    nc = bass.Bass(
        "TRN2", target_bir_lowering=False, debug=False, enable_asserts=True,
        num_devices=NCORES,
    )
    x_own = nc.declare_dram_parameter("x_own", [OWN, D], F32, isOutput=False)
    out = nc.declare_dram_parameter("out", [OWN, D], F32, isOutput=True)

    # row = p*OWN_T + t  -> contiguous 8 KB HBM reads per partition
    xo = x_own.ap().rearrange("(p t) d -> p t d", p=P)
    ov = out.ap().rearrange("(p t) d -> p t d", p=P)

    RG = [list(range(NCORES))]

    DEPTH = 3  # software-pipeline lookahead: phase1(k+DEPTH) before phase2(k)

    with tile.TileContext(nc) as tc, ExitStack() as ctx:
        singles = ctx.enter_context(tc.tile_pool(name="singles", bufs=1))
        xh_pool = ctx.enter_context(tc.tile_pool(name="xh", bufs=16))
        trash_pool = ctx.enter_context(tc.tile_pool(name="tra", bufs=1))
        smalls = ctx.enter_context(tc.tile_pool(name="sm", bufs=DEPTH + 1))
        bigs = ctx.enter_context(tc.tile_pool(name="big", bufs=DEPTH + 1))
        p2tmp = ctx.enter_context(tc.tile_pool(name="p2t", bufs=2))
        dram = ctx.enter_context(
            tc.tile_pool(name="dram", bufs=DEPTH + 1, space="DRAM"))
        psum_g = ctx.enter_context(tc.tile_pool(name="psg", bufs=2, space="PSUM"))
        psum_tr = ctx.enter_context(tc.tile_pool(name="pst", bufs=2, space="PSUM"))
        psum_o = ctx.enter_context(tc.tile_pool(name="pso", bufs=2, space="PSUM"))

        NBUF = DEPTH + 1
        it_bufs = [singles.tile([P, OWN_T, DA], F32, name=f"inbuf{b}")
                   for b in range(NBUF)]
        for b in range(NBUF):
            # ones column, written once; loads only touch [:, :, 0:D]
            nc.gpsimd.memset(it_bufs[b][:, :, D], 1.0)
        ident = singles.tile([P, P], F32)
        make_identity(nc, ident)

        # per-iteration state carried from phase 1 to phase 2
        state = {}

        def phase1(k):
            it = it_bufs[k % NBUF]
            # own-block load, split so phase 1 starts after the first half.
            # Queue discipline: pre-collective DMAs (loads, gin) ride the
            # ACT HWDGE ring; post-collective DMAs (readback, out) ride
            # the SP ring — a ring is FIFO, so a DMA waiting on the
            # collective must never sit ahead of a later phase-1 load.
            nc.scalar.dma_start(out=it[:, 0:OWN_T // 2, 0:D],
                                in_=xo[:, 0:OWN_T // 2, :])
            nc.scalar.dma_start(out=it[:, OWN_T // 2:, 0:D],
                                in_=xo[:, OWN_T // 2:, :])

            # nsq split per engine: ACT squares-with-accum for the front
            # tiles, DVE bn_stats for the rest (one tile from both engines
            # would add cross-engine WAW waits)
            nsq_a = smalls.tile([P, ACT_P1_J], F32, tag="nsq_a")
            stats = smalls.tile([P, OWN_T - ACT_P1_J, 6], F32, tag="stats")
            mv = smalls.tile([P, OWN_T - ACT_P1_J, 2], F32, tag="mv")
            for j in range(OWN_T):
                if j < ACT_P1_J:
                    # dedicated slot per op: a reused slot would add a WAW
                    # semaphore and Activation allows only one wait
                    tr = trash_pool.tile([P, D], F32, tag=f"ta{j}")
                    nc.scalar.activation(
                        out=tr, in_=it[:, j, 0:D], func=AF.Square,
                        accum_out=nsq_a[:, j:j + 1],
                    )
                else:
                    jj = j - ACT_P1_J
                    nc.vector.bn_stats(out=stats[:, jj, :], in_=it[:, j, 0:D])
                    nc.vector.bn_aggr(out=mv[:, jj, :], in_=stats[:, jj, :])
            # nsq_v = D*(var + mean^2); mean^2 << var for gaussian rows
            ymm = smalls.tile([P, OWN_T - ACT_P1_J], F32, tag="ymm")
            nc.vector.tensor_mul(ymm, mv[:, :, 0], mv[:, :, 0])
            yv = smalls.tile([P, OWN_T - ACT_P1_J], F32, tag="yv")
            nc.vector.tensor_add(yv, ymm, mv[:, :, 1])

            # n0 = sqrt(nsq): both sqrts write one n0 tile from one engine
            n0 = smalls.tile([P, OWN_T], F32, tag="n0")
            nc.scalar.activation(out=n0[:, ACT_P1_J:], in_=yv, func=AF.Sqrt,
                                 scale=float(D))
            nc.scalar.activation(out=n0[:, 0:ACT_P1_J], in_=nsq_a,
                                 func=AF.Sqrt)
            r = smalls.tile([P, OWN_T], F32, tag="r")
            nc.vector.reciprocal(r, n0)

            g_ps = [psum_g.tile([P, DA], F32, name=f"g{m}", tag=f"g{m}")
                    for m in range(2)]
            for j in range(OWN_T):
                xh = xh_pool.tile([P, D], F32, tag="xh")
                eng = nc.gpsimd if j < GPS_P2_J else nc.vector
                eng.tensor_scalar(
                    out=xh, in0=it[:, j, 0:D], scalar1=r[:, j:j + 1],
                    scalar2=None, op0=mybir.AluOpType.mult,
                )
                for m in range(2):
                    nc.tensor.matmul(
                        g_ps[m], lhsT=xh[:, m * P:(m + 1) * P], rhs=it[:, j, :],
                        start=(j == 0), stop=(j == OWN_T - 1),
                    )

            # own-block PE-transpose to xT (lhsT for phase 2), after the
            # G' matmuls so PE starts G' as soon as the first tiles land
            xT = [bigs.tile([P, OWN], F32, name=f"xT{dt}", tag=f"xT{dt}")
                  for dt in range(2)]
            for dt in range(2):
                for g in range(2):          # 4 transposes per PSUM bank
                    pst = psum_tr.tile([P, 4 * P], F32, tag="tr")
                    for jj in range(4):
                        j = g * 4 + jj
                        nc.tensor.transpose(
                            pst[:, jj * P:(jj + 1) * P],
                            it[:, j, dt * P:(dt + 1) * P], ident,
                        )
                    nc.scalar.copy(out=xT[dt][:, g * 4 * P:(g + 1) * 4 * P],
                                   in_=pst)

            # ship the G' partial: PSUM -> SBUF -> DRAM bounce -> AllGather
            # (back-to-back 8-core AllGathers pipeline to ~zero marginal
            # cost on this HW, while AllReduce costs ~17.5 us/iter flat —
            # measured with cc_bench.py)
            g_sb = bigs.tile([P, 2 * DA], F32, tag="g_sb")
            for m in range(2):
                nc.scalar.copy(out=g_sb[:, m * DA:(m + 1) * DA], in_=g_ps[m])
            gin = dram.tile([P, 2 * DA], F32, tag="gin")
            gout = dram.tile([NCORES, P, 2 * DA], F32, tag="gout")
            nc.scalar.dma_start(out=gin, in_=g_sb)
            nc.gpsimd.collective_compute(
                "AllGather", mybir.AluOpType.bypass, replica_groups=RG,
                ins=[gin.opt()], outs=[gout.opt()],
            )
            state[k] = (it, xT, gout)

        def phase2(k):
            it, xT, gout = state.pop(k)
            gath = p2tmp.tile([P, NCORES, 2 * DA], F32, tag="gath")
            nc.sync.dma_start(out=gath,
                              in_=gout[:].rearrange("k p f -> p k f"))
            # tree-sum the 8 slabs, split DVE / GpSimd so neither engine
            # eats the whole 7-add chain
            s4 = p2tmp.tile([P, 4, 2 * DA], F32, tag="s4")
            nc.vector.tensor_add(s4[:, 0:2], gath[:, 0:2], gath[:, 4:6])
            nc.gpsimd.tensor_add(s4[:, 2:4], gath[:, 2:4], gath[:, 6:8])
            s2 = p2tmp.tile([P, 2, 2 * DA], F32, tag="s2")
            nc.vector.tensor_add(s2[:, 0:1], s4[:, 0:1], s4[:, 2:3])
            nc.gpsimd.tensor_add(s2[:, 1:2], s4[:, 1:2], s4[:, 3:4])
            gg = p2tmp.tile([P, 2 * DA], F32, tag="gg")
            nc.vector.tensor_add(gg, s2[:, 0], s2[:, 1])

            # own rows x G', then divide by the row sum
            outsb = p2tmp.tile([P, OWN_T, D], F32, tag="outsb")
            for j in range(OWN_T):
                oa = psum_o.tile([P, DA], F32, tag="oa")
                for kk in range(2):
                    nc.tensor.matmul(
                        oa, lhsT=xT[kk][:, j * P:(j + 1) * P],
                        rhs=gg[:, kk * DA:(kk + 1) * DA],
                        start=(kk == 0), stop=(kk == 1),
                    )
                rcp = smalls.tile([P, 1], F32, tag="rcp")
                nc.vector.reciprocal(rcp, oa[:, D:DA])
                nc.vector.tensor_scalar_mul(outsb[:, j, :], oa[:, 0:D], rcp)
            # stores ride the SP ring with the readback: both are
            # post-collective, so neither can stall a pre-collective DMA
            nc.sync.dma_start(out=ov[:, 0:OWN_T // 2, :],
                              in_=outsb[:, 0:OWN_T // 2, :])
            nc.sync.dma_start(out=ov[:, OWN_T // 2:, :],
                              in_=outsb[:, OWN_T // 2:, :])

        for k in range(min(DEPTH, iters)):
            phase1(k)
        for k in range(iters):
            if k + DEPTH < iters:
                phase1(k + DEPTH)
            phase2(k)
    return nc


def _get_nc(iters: int = 1):
    if iters not in _nc_cache:
        nc = _build_nc(iters)
        orig = nc.to_json_bytes
        nc.to_json_bytes = lambda: _legalize_sync_waits(orig())
        _nc_cache[iters] = nc
    return _nc_cache[iters]


LAST_RESULTS = None  # BassKernelResults of the most recent run (for profiling)


def kernel(tensor: np.ndarray, trace: bool = False, **trace_kwargs) -> np.ndarray:
    x = np.ascontiguousarray(np.asarray(tensor, dtype=np.float32))
    assert x.shape == (N, D)
    nc = _get_nc()
    in_maps = [
        {"x_own": np.ascontiguousarray(x[i * OWN:(i + 1) * OWN])}
        for i in range(NCORES)
    ]
    global LAST_RESULTS
    LAST_RESULTS = run_bass_kernel_spmd(
        nc, in_maps, core_ids=list(range(NCORES)), trace=trace, **trace_kwargs
    )
    return np.concatenate([r["out"] for r in LAST_RESULTS.results], axis=0)


# revision 14
# speedup vs baseline: 1.0659x; 1.0659x over previous
"""Trainium2 Bass kernel: cosine-similarity message passing (GNN aggregate).

Math (collapsed — the [N,N] similarity matrix is never materialized):
    x_hat = x / max(||x||, eps)                      row-normalized features
    G'    = x_hat.T @ [x | 1]        [D, D+1]        Gram + column-sum s
    oa    = x @ G'                   [N, D+1]        (query-side normalization
                                                      cancels in the ratio)
    out   = oa[:, :D] / oa[:, D:D+1]

Sharding (v2): G' is a sum over ALL rows, so each core computes a partial
G' from only its OWN 1/8 row block (1 MB read instead of the baseline's
redundant 8 MB full-x stream), and an 8-core AllReduce of the [D, D+1]
partials (263 KB fp32) combines them. Phase 2 (own rows x G', divide by
row sum) is unchanged. Per-core HBM traffic drops 10 MB -> ~2.5 MB; the
collective (~13 us) dominates the steady-state per-iteration time.

Environment quirks encoded here:
 - this walrus build accepts at most ONE sync wait per instruction:
   _legalize_sync_waits hoists extras onto same-engine Drain carriers
   (wired via nc.to_json_bytes); tensor_tensor_reduce doesn't compile.
 - the ACT sqrt spline is near-exact on this HW (a Newton refinement
   measurably HURT accuracy in the baseline session).
 - eps in max(||x||, eps) never binds for gaussian rows (min norm ~14).
"""

import numpy as np
from contextlib import ExitStack

import concourse.bass as bass
import concourse.tile as tile
from concourse import mybir
from concourse.masks import make_identity
from concourse.bass_utils import run_bass_kernel_spmd

N, D = 8192, 256
NCORES = 8
P = 128
OWN = N // NCORES            # 1024 rows per core
OWN_T = OWN // P             # 8 own tiles
DA = D + 1                   # 257: x columns + ones column
F32 = mybir.dt.float32
BF16 = mybir.dt.bfloat16
AF = mybir.ActivationFunctionType

# engine split knobs: tiles j < ACT_P1_J do square+reduce on ScalarE, the
# rest on VectorE; tiles j < GPS_P2_J scale on GpSimd, the rest on VectorE.
ACT_P1_J = 4
GPS_P2_J = 6

_nc_cache = {}


def _legalize_sync_waits(bir_bytes: bytes) -> bytes:
    """This walrus build accepts at most ONE sync wait per instruction.
    Tile emits several; hoist the extras onto same-engine Drain
    instructions placed immediately before (queue order preserves the
    semantics of inline waits)."""
    import orjson
    bir = orjson.loads(bir_bytes)
    ctr = [0]

    def fix_block(blk):
        new_list = []
        for ins in blk.get("instructions", []):
            si = ins.get("sync_info")
            if si:
                waits = si.get("on_wait") or []
                if len(waits) > 1:
                    for w in waits[:-1]:
                        ctr[0] += 1
                        new_list.append({
                            "debug": ins.get("debug", 0),
                            "engine": ins["engine"],
                            "ins": [], "outs": [],
                            "name": f"I-lw{ctr[0]}",
                            "opcode": "Drain",
                            "sync_info": {"on_update": [], "on_wait": [w]},
                        })
                    si["on_wait"] = waits[-1:]
            new_list.append(ins)
        blk["instructions"] = new_list
        for sb in blk.get("blocks", []):
            fix_block(sb)

    for f in bir["functions"]:
        for blk in f["blocks"]:
            fix_block(blk)
    return orjson.dumps(bir)


def _build_nc(iters: int = 1):
    nc = bass.Bass(
        "TRN2", target_bir_lowering=False, debug=False, enable_asserts=True,
        num_devices=NCORES,
    )
    x_own = nc.declare_dram_parameter("x_own", [OWN, D], F32, isOutput=False)
    out = nc.declare_dram_parameter("out", [OWN, D], F32, isOutput=True)

    # row = p*OWN_T + t  -> contiguous 8 KB HBM reads per partition
    xo = x_own.ap().rearrange("(p t) d -> p t d", p=P)
    ov = out.ap().rearrange("(p t) d -> p t d", p=P)

    RG = [list(range(NCORES))]

    DEPTH = 3  # software-pipeline lookahead: phase1(k+DEPTH) before phase2(k)

    with tile.TileContext(nc) as tc, ExitStack() as ctx:
        singles = ctx.enter_context(tc.tile_pool(name="singles", bufs=1))
        xh_pool = ctx.enter_context(tc.tile_pool(name="xh", bufs=16))
        trash_pool = ctx.enter_context(tc.tile_pool(name="tra", bufs=1))
        smalls = ctx.enter_context(tc.tile_pool(name="sm", bufs=DEPTH + 1))
        bigs = ctx.enter_context(tc.tile_pool(name="big", bufs=DEPTH + 1))
        p2tmp = ctx.enter_context(tc.tile_pool(name="p2t", bufs=2))
        dram = ctx.enter_context(
            tc.tile_pool(name="dram", bufs=DEPTH + 1, space="DRAM"))
        psum_g = ctx.enter_context(tc.tile_pool(name="psg", bufs=2, space="PSUM"))
        psum_tr = ctx.enter_context(tc.tile_pool(name="pst", bufs=2, space="PSUM"))
        psum_o = ctx.enter_context(tc.tile_pool(name="pso", bufs=2, space="PSUM"))

        NBUF = DEPTH + 1
        it_bufs = [singles.tile([P, OWN_T, DA], F32, name=f"inbuf{b}")
                   for b in range(NBUF)]
        for b in range(NBUF):
            # ones column, written once; loads only touch [:, :, 0:D]
            nc.gpsimd.memset(it_bufs[b][:, :, D], 1.0)
        ident = singles.tile([P, P], F32)
        make_identity(nc, ident)

        # per-iteration state carried from phase 1 to phase 2
        state = {}

        def phase1(k):
            it = it_bufs[k % NBUF]
            # own-block load, split so phase 1 starts after the first half.
            # Queue discipline: pre-collective DMAs (loads, gin) ride the
            # ACT HWDGE ring; post-collective DMAs (readback, out) ride
            # the SP ring — a ring is FIFO, so a DMA waiting on the
            # collective must never sit ahead of a later phase-1 load.
            nc.scalar.dma_start(out=it[:, 0:OWN_T // 2, 0:D],
                                in_=xo[:, 0:OWN_T // 2, :])
            nc.scalar.dma_start(out=it[:, OWN_T // 2:, 0:D],
                                in_=xo[:, OWN_T // 2:, :])

            # nsq split per engine: ACT squares-with-accum for the front
            # tiles, DVE bn_stats for the rest (one tile from both engines
            # would add cross-engine WAW waits)
            nsq_a = smalls.tile([P, ACT_P1_J], F32, tag="nsq_a")
            stats = smalls.tile([P, OWN_T - ACT_P1_J, 6], F32, tag="stats")
            mv = smalls.tile([P, OWN_T - ACT_P1_J, 2], F32, tag="mv")
            for j in range(OWN_T):
                if j < ACT_P1_J:
                    # dedicated slot per op: a reused slot would add a WAW
                    # semaphore and Activation allows only one wait
                    tr = trash_pool.tile([P, D], F32, tag=f"ta{j}")
                    nc.scalar.activation(
                        out=tr, in_=it[:, j, 0:D], func=AF.Square,
                        accum_out=nsq_a[:, j:j + 1],
                    )
                else:
                    jj = j - ACT_P1_J
                    nc.vector.bn_stats(out=stats[:, jj, :], in_=it[:, j, 0:D])
                    nc.vector.bn_aggr(out=mv[:, jj, :], in_=stats[:, jj, :])
            # nsq_v = D*(var + mean^2); mean^2 << var for gaussian rows
            ymm = smalls.tile([P, OWN_T - ACT_P1_J], F32, tag="ymm")
            nc.vector.tensor_mul(ymm, mv[:, :, 0], mv[:, :, 0])
            yv = smalls.tile([P, OWN_T - ACT_P1_J], F32, tag="yv")
            nc.vector.tensor_add(yv, ymm, mv[:, :, 1])

            # n0 = sqrt(nsq): both sqrts write one n0 tile from one engine
            n0 = smalls.tile([P, OWN_T], F32, tag="n0")
            nc.scalar.activation(out=n0[:, ACT_P1_J:], in_=yv, func=AF.Sqrt,
                                 scale=float(D))
            nc.scalar.activation(out=n0[:, 0:ACT_P1_J], in_=nsq_a,
                                 func=AF.Sqrt)
            r = smalls.tile([P, OWN_T], F32, tag="r")
            nc.vector.reciprocal(r, n0)

            g_ps = [psum_g.tile([P, DA], F32, name=f"g{m}", tag=f"g{m}")
                    for m in range(2)]
            for j in range(OWN_T):
                xh = xh_pool.tile([P, D], F32, tag="xh")
                eng = nc.gpsimd if j < GPS_P2_J else nc.vector
                eng.tensor_scalar(
                    out=xh, in0=it[:, j, 0:D], scalar1=r[:, j:j + 1],
                    scalar2=None, op0=mybir.AluOpType.mult,
                )
                for m in range(2):
                    nc.tensor.matmul(
                        g_ps[m], lhsT=xh[:, m * P:(m + 1) * P], rhs=it[:, j, :],
                        start=(j == 0), stop=(j == OWN_T - 1),
                    )

            # own-block PE-transpose to xT (lhsT for phase 2), after the
            # G' matmuls so PE starts G' as soon as the first tiles land
            xT = [bigs.tile([P, OWN], F32, name=f"xT{dt}", tag=f"xT{dt}")
                  for dt in range(2)]
            for dt in range(2):
                for g in range(2):          # 4 transposes per PSUM bank
                    pst = psum_tr.tile([P, 4 * P], F32, tag="tr")
                    for jj in range(4):
                        j = g * 4 + jj
                        nc.tensor.transpose(
                            pst[:, jj * P:(jj + 1) * P],
                            it[:, j, dt * P:(dt + 1) * P], ident,
                        )
                    nc.scalar.copy(out=xT[dt][:, g * 4 * P:(g + 1) * 4 * P],
                                   in_=pst)

            # ship the G' partial: PSUM -> SBUF -> DRAM bounce -> AllGather
            # (back-to-back 8-core AllGathers pipeline to ~zero marginal
            # cost on this HW, while AllReduce costs ~17.5 us/iter flat —
            # measured with cc_bench.py)
            g_sb = bigs.tile([P, 2 * DA], F32, tag="g_sb")
            for m in range(2):
                nc.scalar.copy(out=g_sb[:, m * DA:(m + 1) * DA], in_=g_ps[m])
            gin = dram.tile([P, 2 * DA], F32, tag="gin")
            # Shared scratchpad output: the AllGather writes ONE copy for
            # the whole chip instead of one per core (8x less SDMA/HBM
            # write traffic from the collective)
            gout = dram.tile([NCORES, P, 2 * DA], F32, tag="gout",
                             addr_space="Shared")
            nc.scalar.dma_start(out=gin, in_=g_sb)
            nc.gpsimd.collective_compute(
                "AllGather", mybir.AluOpType.bypass, replica_groups=RG,
                ins=[gin.opt()], outs=[gout.opt()],
            )
            state[k] = (it, xT, gout)

        def phase2(k):
            it, xT, gout = state.pop(k)
            gath = p2tmp.tile([P, NCORES, 2 * DA], F32, tag="gath")
            nc.sync.dma_start(out=gath,
                              in_=gout[:].rearrange("k p f -> p k f"))
            # tree-sum the 8 slabs, split DVE / GpSimd so neither engine
            # eats the whole 7-add chain
            s4 = p2tmp.tile([P, 4, 2 * DA], F32, tag="s4")
            nc.vector.tensor_add(s4[:, 0:2], gath[:, 0:2], gath[:, 4:6])
            nc.gpsimd.tensor_add(s4[:, 2:4], gath[:, 2:4], gath[:, 6:8])
            s2 = p2tmp.tile([P, 2, 2 * DA], F32, tag="s2")
            nc.vector.tensor_add(s2[:, 0:1], s4[:, 0:1], s4[:, 2:3])
            nc.gpsimd.tensor_add(s2[:, 1:2], s4[:, 1:2], s4[:, 3:4])
            gg = p2tmp.tile([P, 2 * DA], F32, tag="gg")
            nc.vector.tensor_add(gg, s2[:, 0], s2[:, 1])

            # own rows x G', then divide by the row sum
            outsb = p2tmp.tile([P, OWN_T, D], F32, tag="outsb")
            for j in range(OWN_T):
                oa = psum_o.tile([P, DA], F32, tag="oa")
                for kk in range(2):
                    nc.tensor.matmul(
                        oa, lhsT=xT[kk][:, j * P:(j + 1) * P],
                        rhs=gg[:, kk * DA:(kk + 1) * DA],
                        start=(kk == 0), stop=(kk == 1),
                    )
                rcp = smalls.tile([P, 1], F32, tag="rcp")
                nc.vector.reciprocal(rcp, oa[:, D:DA])
                nc.vector.tensor_scalar_mul(outsb[:, j, :], oa[:, 0:D], rcp)
            # stores ride the SP ring with the readback: both are
            # post-collective, so neither can stall a pre-collective DMA
            nc.sync.dma_start(out=ov[:, 0:OWN_T // 2, :],
                              in_=outsb[:, 0:OWN_T // 2, :])
            nc.sync.dma_start(out=ov[:, OWN_T // 2:, :],
                              in_=outsb[:, OWN_T // 2:, :])

        for k in range(min(DEPTH, iters)):
            phase1(k)
        for k in range(iters):
            if k + DEPTH < iters:
                phase1(k + DEPTH)
            phase2(k)
    return nc


def _get_nc(iters: int = 1):
    if iters not in _nc_cache:
        nc = _build_nc(iters)
        orig = nc.to_json_bytes
        nc.to_json_bytes = lambda: _legalize_sync_waits(orig())
        _nc_cache[iters] = nc
    return _nc_cache[iters]


LAST_RESULTS = None  # BassKernelResults of the most recent run (for profiling)


def kernel(tensor: np.ndarray, trace: bool = False, **trace_kwargs) -> np.ndarray:
    x = np.ascontiguousarray(np.asarray(tensor, dtype=np.float32))
    assert x.shape == (N, D)
    nc = _get_nc()
    in_maps = [
        {"x_own": np.ascontiguousarray(x[i * OWN:(i + 1) * OWN])}
        for i in range(NCORES)
    ]
    global LAST_RESULTS
    LAST_RESULTS = run_bass_kernel_spmd(
        nc, in_maps, core_ids=list(range(NCORES)), trace=trace, **trace_kwargs
    )
    return np.concatenate([r["out"] for r in LAST_RESULTS.results], axis=0)


# revision 16
# speedup vs baseline: 1.5180x; 1.4241x over previous
"""Trainium2 Bass kernel: cosine-similarity message passing (GNN aggregate).

Math (collapsed — the [N,N] similarity matrix is never materialized):
    x_hat = x / max(||x||, eps)                      row-normalized features
    G'    = x_hat.T @ [x | 1]        [D, D+1]        Gram + column-sum s
    oa    = x @ G'                   [N, D+1]        (query-side normalization
                                                      cancels in the ratio)
    out   = oa[:, :D] / oa[:, D:D+1]

Sharding (v2): G' is a sum over ALL rows, so each core computes a partial
G' from only its OWN 1/8 row block (1 MB read instead of the baseline's
redundant 8 MB full-x stream), and an 8-core AllReduce of the [D, D+1]
partials (263 KB fp32) combines them. Phase 2 (own rows x G', divide by
row sum) is unchanged. Per-core HBM traffic drops 10 MB -> ~2.5 MB; the
collective (~13 us) dominates the steady-state per-iteration time.

Environment quirks encoded here:
 - this walrus build accepts at most ONE sync wait per instruction:
   _legalize_sync_waits hoists extras onto same-engine Drain carriers
   (wired via nc.to_json_bytes); tensor_tensor_reduce doesn't compile.
 - the ACT sqrt spline is near-exact on this HW (a Newton refinement
   measurably HURT accuracy in the baseline session).
 - eps in max(||x||, eps) never binds for gaussian rows (min norm ~14).
"""

import numpy as np
from contextlib import ExitStack

import concourse.bass as bass
import concourse.tile as tile
from concourse import mybir
from concourse.masks import make_identity
from concourse.bass_utils import run_bass_kernel_spmd

N, D = 8192, 256
NCORES = 8
P = 128
OWN = N // NCORES            # 1024 rows per core
OWN_T = OWN // P             # 8 own tiles
DA = D + 1                   # 257: x columns + ones column
F32 = mybir.dt.float32
BF16 = mybir.dt.bfloat16
AF = mybir.ActivationFunctionType

# engine split knobs: tiles j < ACT_P1_J do square+reduce on ScalarE, the
# rest on VectorE; tiles j < GPS_P2_J scale on GpSimd, the rest on VectorE.
ACT_P1_J = 4
GPS_P2_J = 6
CC_KIND = "AR"               # "AG" (gather + local tree-sum) or "AR"

_nc_cache = {}


def _legalize_sync_waits(bir_bytes: bytes) -> bytes:
    """This walrus build accepts at most ONE sync wait per instruction.
    Tile emits several; hoist the extras onto same-engine Drain
    instructions placed immediately before (queue order preserves the
    semantics of inline waits)."""
    import orjson
    bir = orjson.loads(bir_bytes)
    ctr = [0]

    def fix_block(blk):
        new_list = []
        for ins in blk.get("instructions", []):
            si = ins.get("sync_info")
            if si:
                waits = si.get("on_wait") or []
                if len(waits) > 1:
                    for w in waits[:-1]:
                        ctr[0] += 1
                        new_list.append({
                            "debug": ins.get("debug", 0),
                            "engine": ins["engine"],
                            "ins": [], "outs": [],
                            "name": f"I-lw{ctr[0]}",
                            "opcode": "Drain",
                            "sync_info": {"on_update": [], "on_wait": [w]},
                        })
                    si["on_wait"] = waits[-1:]
            new_list.append(ins)
        blk["instructions"] = new_list
        for sb in blk.get("blocks", []):
            fix_block(sb)

    for f in bir["functions"]:
        for blk in f["blocks"]:
            fix_block(blk)
    return orjson.dumps(bir)


def _build_nc(iters: int = 1):
    nc = bass.Bass(
        "TRN2", target_bir_lowering=False, debug=False, enable_asserts=True,
        num_devices=NCORES,
    )
    x_own = nc.declare_dram_parameter("x_own", [OWN, D], F32, isOutput=False)
    out = nc.declare_dram_parameter("out", [OWN, D], F32, isOutput=True)

    # row = p*OWN_T + t  -> contiguous 8 KB HBM reads per partition
    xo = x_own.ap().rearrange("(p t) d -> p t d", p=P)
    ov = out.ap().rearrange("(p t) d -> p t d", p=P)

    RG = [list(range(NCORES))]

    DEPTH = 3  # software-pipeline lookahead: phase1(k+DEPTH) before phase2(k)

    with tile.TileContext(nc) as tc, ExitStack() as ctx:
        singles = ctx.enter_context(tc.tile_pool(name="singles", bufs=1))
        xh_pool = ctx.enter_context(tc.tile_pool(name="xh", bufs=16))
        trash_pool = ctx.enter_context(tc.tile_pool(name="tra", bufs=1))
        smalls = ctx.enter_context(tc.tile_pool(name="sm", bufs=DEPTH + 1))
        bigs = ctx.enter_context(tc.tile_pool(name="big", bufs=DEPTH + 1))
        p2tmp = ctx.enter_context(tc.tile_pool(name="p2t", bufs=2))
        dram = ctx.enter_context(
            tc.tile_pool(name="dram", bufs=DEPTH + 1, space="DRAM"))
        psum_g = ctx.enter_context(tc.tile_pool(name="psg", bufs=2, space="PSUM"))
        psum_tr = ctx.enter_context(tc.tile_pool(name="pst", bufs=2, space="PSUM"))
        psum_o = ctx.enter_context(tc.tile_pool(name="pso", bufs=2, space="PSUM"))

        NBUF = DEPTH + 1
        it_bufs = [singles.tile([P, OWN_T, DA], F32, name=f"inbuf{b}")
                   for b in range(NBUF)]
        for b in range(NBUF):
            # ones column, written once; loads only touch [:, :, 0:D]
            nc.gpsimd.memset(it_bufs[b][:, :, D], 1.0)
        ident = singles.tile([P, P], F32)
        make_identity(nc, ident)

        # per-iteration state carried from phase 1 to phase 2
        state = {}

        def phase1(k):
            it = it_bufs[k % NBUF]
            # own-block load, split so phase 1 starts after the first half.
            # Queue discipline: pre-collective DMAs (loads, gin) ride the
            # ACT HWDGE ring; post-collective DMAs (readback, out) ride
            # the SP ring — a ring is FIFO, so a DMA waiting on the
            # collective must never sit ahead of a later phase-1 load.
            nc.scalar.dma_start(out=it[:, 0:OWN_T // 2, 0:D],
                                in_=xo[:, 0:OWN_T // 2, :])
            nc.scalar.dma_start(out=it[:, OWN_T // 2:, 0:D],
                                in_=xo[:, OWN_T // 2:, :])

            # nsq split per engine: ACT squares-with-accum for the front
            # tiles, DVE bn_stats for the rest (one tile from both engines
            # would add cross-engine WAW waits)
            nsq_a = smalls.tile([P, ACT_P1_J], F32, tag="nsq_a")
            stats = smalls.tile([P, OWN_T - ACT_P1_J, 6], F32, tag="stats")
            mv = smalls.tile([P, OWN_T - ACT_P1_J, 2], F32, tag="mv")
            for j in range(OWN_T):
                if j < ACT_P1_J:
                    # dedicated slot per op: a reused slot would add a WAW
                    # semaphore and Activation allows only one wait
                    tr = trash_pool.tile([P, D], F32, tag=f"ta{j}")
                    nc.scalar.activation(
                        out=tr, in_=it[:, j, 0:D], func=AF.Square,
                        accum_out=nsq_a[:, j:j + 1],
                    )
                else:
                    jj = j - ACT_P1_J
                    nc.vector.bn_stats(out=stats[:, jj, :], in_=it[:, j, 0:D])
                    nc.vector.bn_aggr(out=mv[:, jj, :], in_=stats[:, jj, :])
            # nsq_v = D*(var + mean^2); mean^2 << var for gaussian rows
            ymm = smalls.tile([P, OWN_T - ACT_P1_J], F32, tag="ymm")
            nc.vector.tensor_mul(ymm, mv[:, :, 0], mv[:, :, 0])
            yv = smalls.tile([P, OWN_T - ACT_P1_J], F32, tag="yv")
            nc.vector.tensor_add(yv, ymm, mv[:, :, 1])

            # n0 = sqrt(nsq): both sqrts write one n0 tile from one engine
            n0 = smalls.tile([P, OWN_T], F32, tag="n0")
            nc.scalar.activation(out=n0[:, ACT_P1_J:], in_=yv, func=AF.Sqrt,
                                 scale=float(D))
            nc.scalar.activation(out=n0[:, 0:ACT_P1_J], in_=nsq_a,
                                 func=AF.Sqrt)
            r = smalls.tile([P, OWN_T], F32, tag="r")
            nc.vector.reciprocal(r, n0)

            g_ps = [psum_g.tile([P, DA], F32, name=f"g{m}", tag=f"g{m}")
                    for m in range(2)]
            for j in range(OWN_T):
                xh = xh_pool.tile([P, D], F32, tag="xh")
                eng = nc.gpsimd if j < GPS_P2_J else nc.vector
                eng.tensor_scalar(
                    out=xh, in0=it[:, j, 0:D], scalar1=r[:, j:j + 1],
                    scalar2=None, op0=mybir.AluOpType.mult,
                )
                for m in range(2):
                    nc.tensor.matmul(
                        g_ps[m], lhsT=xh[:, m * P:(m + 1) * P], rhs=it[:, j, :],
                        start=(j == 0), stop=(j == OWN_T - 1),
                    )

            # own-block PE-transpose to xT (lhsT for phase 2), after the
            # G' matmuls so PE starts G' as soon as the first tiles land
            xT = [bigs.tile([P, OWN], F32, name=f"xT{dt}", tag=f"xT{dt}")
                  for dt in range(2)]
            for dt in range(2):
                for g in range(2):          # 4 transposes per PSUM bank
                    pst = psum_tr.tile([P, 4 * P], F32, tag="tr")
                    for jj in range(4):
                        j = g * 4 + jj
                        nc.tensor.transpose(
                            pst[:, jj * P:(jj + 1) * P],
                            it[:, j, dt * P:(dt + 1) * P], ident,
                        )
                    nc.scalar.copy(out=xT[dt][:, g * 4 * P:(g + 1) * 4 * P],
                                   in_=pst)

            # ship the G' partial: PSUM -> SBUF -> DRAM bounce -> AllGather
            # (back-to-back 8-core AllGathers pipeline to ~zero marginal
            # cost on this HW, while AllReduce costs ~17.5 us/iter flat —
            # measured with cc_bench.py)
            g_sb = bigs.tile([P, 2 * DA], F32, tag="g_sb")
            for m in range(2):
                nc.scalar.copy(out=g_sb[:, m * DA:(m + 1) * DA], in_=g_ps[m])
            gin = dram.tile([P, 2 * DA], F32, tag="gin")
            nc.scalar.dma_start(out=gin, in_=g_sb)
            if CC_KIND == "AG":
                # Shared scratchpad output: the AllGather writes ONE copy
                # for the whole chip instead of one per core
                gout = dram.tile([NCORES, P, 2 * DA], F32, tag="gout",
                                 addr_space="Shared")
                nc.gpsimd.collective_compute(
                    "AllGather", mybir.AluOpType.bypass, replica_groups=RG,
                    ins=[gin.opt()], outs=[gout.opt()],
                )
            else:
                # AllReduce: pricier on the collective device but the
                # readback is 263 KB instead of 2.1 MB — kernel DMA traffic
                # inflates the in-flight collective (measured: AG+4MB DMA
                # = 27 us/iter vs 12 standalone), so fewer bytes win
                gout = dram.tile([P, 2 * DA], F32, tag="gout")
                nc.gpsimd.collective_compute(
                    "AllReduce", mybir.AluOpType.add, replica_groups=RG,
                    ins=[gin.opt()], outs=[gout.opt()],
                )
            state[k] = (it, xT, gout)

        def phase2(k):
            it, xT, gout = state.pop(k)
            if CC_KIND == "AG":
                gath = p2tmp.tile([P, NCORES, 2 * DA], F32, tag="gath")
                nc.sync.dma_start(out=gath,
                                  in_=gout[:].rearrange("k p f -> p k f"))
                # tree-sum the 8 slabs, split DVE / GpSimd so neither
                # engine eats the whole 7-add chain
                s4 = p2tmp.tile([P, 4, 2 * DA], F32, tag="s4")
                nc.vector.tensor_add(s4[:, 0:2], gath[:, 0:2], gath[:, 4:6])
                nc.gpsimd.tensor_add(s4[:, 2:4], gath[:, 2:4], gath[:, 6:8])
                s2 = p2tmp.tile([P, 2, 2 * DA], F32, tag="s2")
                nc.vector.tensor_add(s2[:, 0:1], s4[:, 0:1], s4[:, 2:3])
                nc.gpsimd.tensor_add(s2[:, 1:2], s4[:, 1:2], s4[:, 3:4])
                gg = p2tmp.tile([P, 2 * DA], F32, tag="gg")
                nc.vector.tensor_add(gg, s2[:, 0], s2[:, 1])
            else:
                gg = p2tmp.tile([P, 2 * DA], F32, tag="gg")
                nc.sync.dma_start(out=gg, in_=gout)

            # own rows x G', then divide by the row sum
            outsb = p2tmp.tile([P, OWN_T, D], F32, tag="outsb")
            for j in range(OWN_T):
                oa = psum_o.tile([P, DA], F32, tag="oa")
                for kk in range(2):
                    nc.tensor.matmul(
                        oa, lhsT=xT[kk][:, j * P:(j + 1) * P],
                        rhs=gg[:, kk * DA:(kk + 1) * DA],
                        start=(kk == 0), stop=(kk == 1),
                    )
                rcp = smalls.tile([P, 1], F32, tag="rcp")
                nc.vector.reciprocal(rcp, oa[:, D:DA])
                nc.vector.tensor_scalar_mul(outsb[:, j, :], oa[:, 0:D], rcp)
            # stores ride the SP ring with the readback: both are
            # post-collective, so neither can stall a pre-collective DMA
            nc.sync.dma_start(out=ov[:, 0:OWN_T // 2, :],
                              in_=outsb[:, 0:OWN_T // 2, :])
            nc.sync.dma_start(out=ov[:, OWN_T // 2:, :],
                              in_=outsb[:, OWN_T // 2:, :])

        for k in range(min(DEPTH, iters)):
            phase1(k)
        for k in range(iters):
            if k + DEPTH < iters:
                phase1(k + DEPTH)
            phase2(k)
    return nc


def _get_nc(iters: int = 1):
    if iters not in _nc_cache:
        nc = _build_nc(iters)
        orig = nc.to_json_bytes
        nc.to_json_bytes = lambda: _legalize_sync_waits(orig())
        _nc_cache[iters] = nc
    return _nc_cache[iters]


LAST_RESULTS = None  # BassKernelResults of the most recent run (for profiling)


def kernel(tensor: np.ndarray, trace: bool = False, **trace_kwargs) -> np.ndarray:
    x = np.ascontiguousarray(np.asarray(tensor, dtype=np.float32))
    assert x.shape == (N, D)
    nc = _get_nc()
    in_maps = [
        {"x_own": np.ascontiguousarray(x[i * OWN:(i + 1) * OWN])}
        for i in range(NCORES)
    ]
    global LAST_RESULTS
    LAST_RESULTS = run_bass_kernel_spmd(
        nc, in_maps, core_ids=list(range(NCORES)), trace=trace, **trace_kwargs
    )
    return np.concatenate([r["out"] for r in LAST_RESULTS.results], axis=0)
